# revision 2
# baseline (speedup 1.0000x reference)
"""BinarizedLeNet5/CIFAR10 Trainium2 kernel (8-core data parallel), v2.

vs baseline: fp8 staging for all binary tensors (sp/ic2/s2all/fc1in/s3 and
the sign-binarized weights), conv2 col-packed into [128,1024] psum tiles
(two 4-sample groups via tile_position=(0,64)), DMA queues spread across
SP/ACT/Pool, repack collapsed to 4 fat DMAs via a K-tile permutation
(kt=yx//2, kp=2c+yx%2) with the matching fc1w host layout and an
un-permuting output DMA, pad-only memsets, fc weights preloaded on the
Pool queue during early chunks, and an Exp,Exp,Ln,Ln tail that needs one
activation-table reload instead of three.
"""
import sys
import numpy as np

sys.path.insert(0, "/opt/pypackages")
sys.path.insert(0, "/opt/trn_rl_repo")

import ml_dtypes

BF = ml_dtypes.bfloat16
F8 = ml_dtypes.float8_e4m3
NCORES = 8
B = 2048
BC = B // NCORES          # 256 samples per core
CH = 16                   # samples per chunk
NCHUNK = BC // CH         # 16 chunks
EPS = np.float32(1e-5)

_nc_cache = {}


def _f32(x):
    return np.asarray(x, np.float32)


def _host_prep(inputs):
    """Build all per-core device input arrays."""
    x = _f32(inputs["x"])                      # [2048,3,32,32]

    # ---- conv1 im2col, hi/lo bf16 ----
    xhi = x.astype(BF)
    xlo = (x - xhi.astype(np.float32)).astype(BF)

    def im2col(xq):
        xp = np.zeros((B, 3, 34, 34), BF)
        xp[:, :, 1:33, 1:33] = xq
        ic = np.zeros((B, 128, 8, 32), BF)
        for j in range(4):
            for dy in range(3):
                for dx in range(3):
                    t = 3 * dy + dx
                    ic[:, 32 * j + 3 * t:32 * j + 3 * t + 3] = \
                        xp[:, :, 8 * j + dy:8 * j + dy + 8, dx:dx + 32]
        ic = ic.reshape(NCORES, NCHUNK, CH, 128, 8, 32)
        ic = np.ascontiguousarray(ic.transpose(0, 1, 3, 2, 4, 5))
        return ic.reshape(NCORES, NCHUNK, 128, CH * 256)

    ic1h = im2col(xhi)
    ic1l = im2col(xlo)

    # ---- conv1 stationary: block-diag, k = 3*(3dy+dx)+c, out p = 4*co+j ----
    w1s = np.sign(_f32(inputs["conv1_w"]))               # [32,3,3,3]
    w1k = np.ascontiguousarray(w1s.transpose(2, 3, 1, 0)).reshape(27, 32)
    w1_st = np.zeros((128, 128), BF)
    for j in range(4):
        for co in range(32):
            w1_st[32 * j:32 * j + 27, 4 * co + j] = w1k[:, co].astype(BF)

    # ---- conv1 sign ACT constants (bias folded), indexed p = 4*co+j ----
    inv1 = _f32(inputs["bn1_g"]) / np.sqrt(_f32(inputs["bn1_v"]) + EPS)
    sh1c = (_f32(inputs["conv1_b"]) - _f32(inputs["bn1_m"])) * inv1 \
        + _f32(inputs["bn1_b"])
    sc1 = np.repeat(inv1, 4).reshape(128, 1).astype(np.float32)
    sh1 = np.repeat(sh1c, 4).reshape(128, 1).astype(np.float32)

    # ---- conv2 stationaries [3][96,64] fp8: p = 32*dx + c ----
    w2s = np.sign(_f32(inputs["conv2_w"]))               # [64,32,3,3]
    w2_st = np.zeros((3, 96, 64), F8)
    for dy in range(3):
        for dx in range(3):
            w2_st[dy, 32 * dx:32 * dx + 32] = w2s[:, :, dy, dx].T.astype(F8)

    # ---- conv2 post-pool sign constants, partition (g2, c64) ----
    inv2 = _f32(inputs["bn2_g"]) / np.sqrt(_f32(inputs["bn2_v"]) + EPS)
    sh2c = _f32(inputs["bn2_b"]) - _f32(inputs["bn2_m"]) * inv2
    sc2v = inv2
    sh2v = _f32(inputs["conv2_b"]) * inv2 + sh2c
    sc2 = np.tile(sc2v, 2).reshape(128, 1).astype(np.float32)
    sh2 = np.tile(sh2v, 2).reshape(128, 1).astype(np.float32)

    # ---- fc1: K permutation kt = yx//2, kp = c + 64*(yx%2) ----
    fw1 = np.sign(_f32(inputs["fc1_w"]))                 # [512,4096]
    A = fw1.T.reshape(64, 64, 512)                       # [c][yx][m]
    Bm = A.reshape(64, 32, 2, 512)                       # [c][kt][yxp][m]
    fc1_st = np.ascontiguousarray(
        Bm.transpose(1, 2, 0, 3).reshape(32, 128, 512).astype(F8))  # [kt][kp][m]
    inv3 = _f32(inputs["bn3_g"]) / np.sqrt(_f32(inputs["bn3_v"]) + EPS)
    sh3c = (_f32(inputs["fc1_b"]) - _f32(inputs["bn3_m"])) * inv3 \
        + _f32(inputs["bn3_b"])
    sc3 = np.ascontiguousarray(inv3.reshape(4, 128).T).astype(np.float32)   # [128,4]
    sh3 = np.ascontiguousarray(sh3c.reshape(4, 128).T).astype(np.float32)

    # ---- fc2 ----
    fw2 = np.sign(_f32(inputs["fc2_w"]))                 # [256,512]
    fc2_st = np.ascontiguousarray(fw2.T.reshape(4, 128, 256).astype(F8))
    inv4 = _f32(inputs["bn4_g"]) / np.sqrt(_f32(inputs["bn4_v"]) + EPS)
    sh4c = (_f32(inputs["fc2_b"]) - _f32(inputs["bn4_m"])) * inv4 \
        + _f32(inputs["bn4_b"])
    sc4 = np.ascontiguousarray(inv4.reshape(2, 128).T).astype(np.float32)   # [128,2]
    sh4 = np.ascontiguousarray(sh4c.reshape(2, 128).T).astype(np.float32)

    # ---- fc3 hi/lo ----
    w3 = _f32(inputs["fc3_w"]).T                         # [256,10]
    w3h = w3.astype(BF)
    w3l = (w3 - w3h.astype(np.float32)).astype(BF)
    w3h = np.ascontiguousarray(w3h.reshape(2, 128, 10))
    w3l = np.ascontiguousarray(w3l.reshape(2, 128, 10))
    b3bc = np.tile(_f32(inputs["fc3_b"]).reshape(1, 10), (128, 1)).astype(np.float32)

    shared = dict(w1=w1_st, w2=w2_st, fc1w=fc1_st, fc2w=fc2_st,
                  w3h=w3h, w3l=w3l, sc1=sc1, sh1=sh1, sc2=sc2, sh2=sh2,
                  sc3=sc3, sh3=sh3, sc4=sc4, sh4=sh4, b3bc=b3bc)
    in_maps = []
    for ci in range(NCORES):
        m = dict(shared)
        m["ic1h"] = np.ascontiguousarray(ic1h[ci])
        m["ic1l"] = np.ascontiguousarray(ic1l[ci])
        in_maps.append(m)
    return in_maps


def _build_module(reps=1):
    import concourse.bass as bass
    import concourse.mybir as mybir
    import concourse.tile as tile
    from concourse import bacc
    from contextlib import ExitStack

    F32 = mybir.dt.float32
    BF16 = mybir.dt.bfloat16
    FP8 = mybir.dt.float8e4
    AF = mybir.ActivationFunctionType
    ALU = mybir.AluOpType

    nc = bacc.Bacc("TRN2", target_bir_lowering=False, debug=False)

    # ---- DRAM tensors ----
    d_ic1h = nc.dram_tensor("ic1h", [NCHUNK, 128, CH * 256], BF16, kind="ExternalInput")
    d_ic1l = nc.dram_tensor("ic1l", [NCHUNK, 128, CH * 256], BF16, kind="ExternalInput")
    d_w1 = nc.dram_tensor("w1", [128, 128], BF16, kind="ExternalInput")
    d_w2 = nc.dram_tensor("w2", [3, 96, 64], FP8, kind="ExternalInput")
    d_fc1w = nc.dram_tensor("fc1w", [32, 128, 512], FP8, kind="ExternalInput")
    d_fc2w = nc.dram_tensor("fc2w", [4, 128, 256], FP8, kind="ExternalInput")
    d_w3h = nc.dram_tensor("w3h", [2, 128, 10], BF16, kind="ExternalInput")
    d_w3l = nc.dram_tensor("w3l", [2, 128, 10], BF16, kind="ExternalInput")
    d_sc1 = nc.dram_tensor("sc1", [128, 1], F32, kind="ExternalInput")
    d_sh1 = nc.dram_tensor("sh1", [128, 1], F32, kind="ExternalInput")
    d_sc2 = nc.dram_tensor("sc2", [128, 1], F32, kind="ExternalInput")
    d_sh2 = nc.dram_tensor("sh2", [128, 1], F32, kind="ExternalInput")
    d_sc3 = nc.dram_tensor("sc3", [128, 4], F32, kind="ExternalInput")
    d_sh3 = nc.dram_tensor("sh3", [128, 4], F32, kind="ExternalInput")
    d_sc4 = nc.dram_tensor("sc4", [128, 2], F32, kind="ExternalInput")
    d_sh4 = nc.dram_tensor("sh4", [128, 2], F32, kind="ExternalInput")
    d_b3bc = nc.dram_tensor("b3bc", [128, 10], F32, kind="ExternalInput")
    d_out = nc.dram_tensor("out", [BC, 10], F32, kind="ExternalOutput")

    NBUF = 3                       # staging ping-pong depth
    SPW = 4 * CH * 18              # sp payload per partition
    SP_FREE = SPW + 8
    IC2_FREE = 18 * CH * 18 + 8    # (R 18, s CH, W 18)

    with tile.TileContext(nc) as tc, ExitStack() as ctx:
        const = ctx.enter_context(tc.tile_pool(name="const", bufs=1))
        icp = ctx.enter_context(tc.tile_pool(name="icp", bufs=2))
        wk = ctx.enter_context(tc.tile_pool(name="wk", bufs=3))
        pp1 = ctx.enter_context(tc.tile_pool(name="pp1", bufs=4, space="PSUM"))
        pp2 = ctx.enter_context(tc.tile_pool(name="pp2", bufs=2, space="PSUM"))

        # ---- persistent tiles ----
        w1_sb = const.tile([128, 128], BF16, tag="w1")
        w2_sb = const.tile([96, 3, 64], FP8, tag="w2")
        fc1w_sb = const.tile([128, 32, 512], FP8, tag="fc1w")
        fc2w_sb = const.tile([128, 4, 256], FP8, tag="fc2w")
        w3h_sb = const.tile([128, 2, 10], BF16, tag="w3h")
        w3l_sb = const.tile([128, 2, 10], BF16, tag="w3l")
        sc1_sb = const.tile([128, 1], F32, tag="sc1")
        sh1_sb = const.tile([128, 1], F32, tag="sh1")
        sc2_sb = const.tile([128, 1], F32, tag="sc2")
        sh2_sb = const.tile([128, 1], F32, tag="sh2")
        sc3_sb = const.tile([128, 4], F32, tag="sc3")
        sh3_sb = const.tile([128, 4], F32, tag="sh3")
        sc4_sb = const.tile([128, 2], F32, tag="sc4")
        sh4_sb = const.tile([128, 2], F32, tag="sh4")
        b3bc_sb = const.tile([128, 10], F32, tag="b3bc")
        sp_t = [const.tile([128, SP_FREE], FP8, tag=f"sp{i}", name=f"sp{i}")
                for i in range(NBUF)]
        ic2_t = [const.tile([96, IC2_FREE], FP8, tag=f"ic2_{i}", name=f"ic2_{i}")
                 for i in range(NBUF)]
        # s2all: partition (g2, c64), free = yx*128 + sh,
        #   sh = (2*chk + tau)*4 + sl,  sample s = 8*(sh//4) + 4g + sh%4
        s2all = const.tile([128, 64 * 128], FP8, tag="s2all")
        # fc1in: partition kp = c + 64*(yx%2), free = kt*BC + (g*128 + sh)
        fc1in = const.tile([128, 32 * BC], FP8, tag="fc1in")
        s3_sb = const.tile([128, 4, BC], FP8, tag="s3")
        u4_sb = const.tile([128, 2, BC], F32, tag="u4")
        s4h_sb = const.tile([128, 2, BC], BF16, tag="s4h")
        s4l_sb = const.tile([128, 2, BC], BF16, tag="s4l")
        s4r_sb = const.tile([128, 2, BC], F32, tag="s4r")

        def ap_of(t, dims, off=0):
            return bass.AP(tensor=t.tensor, offset=t.offset + off,
                           ap=[list(t.ap[0])] + [list(d) for d in dims])

        # ---- setup: critical-path weights first, big fc weights on Pool ----
        nc.sync.dma_start(w1_sb[:], d_w1.ap())
        for dy in range(3):
            nc.gpsimd.dma_start(w2_sb[:, dy, :], d_w2.ap()[dy])
        for t, d in [(sc1_sb, d_sc1), (sh1_sb, d_sh1)]:
            nc.sync.dma_start(t[:], d.ap())
        for t, d in [(sc2_sb, d_sc2), (sh2_sb, d_sh2)]:
            nc.scalar.dma_start(t[:], d.ap())

        # zero ONLY the pad regions (they stay zero forever):
        # sp pads: cols w==0 and w==17 of each 18-block, plus the 8 slack cols
        for t in sp_t:
            nc.vector.memset(
                ap_of(t, [[18, SPW // 18], [17, 2], [1, 1]]), 0.0)
            nc.vector.memset(ap_of(t, [[1, 8]], SPW), 0.0)
        # ic2 pads: halo rows R=0 and R=17 (+ slack)
        for t in ic2_t:
            nc.vector.memset(
                ap_of(t, [[17 * CH * 18, 2], [1, CH * 18]]), 0.0)
            nc.vector.memset(ap_of(t, [[1, 8]], 18 * CH * 18), 0.0)

        for _rep in range(reps):
            # ================= chunk loop (software-pipelined) =================
            # iteration k emits conv1 of chunk k, then conv2 of chunk k-1, so
            # the in-order PE queue never stalls on chunk k-1's ic2 DMAs.
            def conv1_part(chk):
                sp = sp_t[chk % NBUF]
                ic2 = ic2_t[chk % NBUF]

                ich = icp.tile([128, CH * 256], BF16, tag="ich")
                nc.sync.dma_start(ich[:], d_ic1h.ap()[chk])
                icl = icp.tile([128, CH * 256], BF16, tag="icl")
                nc.scalar.dma_start(icl[:], d_ic1l.ap()[chk])

                # ---- conv1: 8 one-bank psum tiles of 2 samples ----
                for t in range(8):
                    p1 = pp1.tile([128, 512], F32, tag="c1")
                    sl = bass.ds(t * 512, 512)
                    nc.tensor.matmul(p1[:], w1_sb[:], ich[:, sl],
                                     start=True, stop=False)
                    nc.tensor.matmul(p1[:], w1_sb[:], icl[:, sl],
                                     start=False, stop=True)
                    # DVE: 2x2 maxpool in ONE XY-window reduce from psum
                    pl1 = wk.tile([128, 128], F32, tag="pl1")
                    nc.vector.tensor_reduce(
                        ap_of(pl1, [[16, 8], [1, 16]]),
                        ap_of(p1, [[64, 8], [2, 16], [32, 2], [1, 2]]),
                        mybir.AxisListType.XY, ALU.max)
                    # ACT: sign(bn1) -> +-1 fp8 straight into sp
                    nc.scalar.activation(
                        ap_of(sp, [[18, 2], [CH * 18, 4], [1, 16]],
                              (2 * t) * 18 + 1),
                        pl1[:], AF.Sign, bias=sh1_sb[:], scale=sc1_sb[:])

                # ---- ic2 build: 3 contiguous-run DMAs (SP, ACT, Pool) ----
                RUN = 4 * CH * 18
                for dx in range(3):
                    src = bass.AP(tensor=sp.tensor, offset=sp.offset + dx,
                                  ap=[list(sp.ap[0]), [1, RUN]])
                    dst_t = ic2[32 * dx:32 * (dx + 1)]
                    dst = bass.AP(tensor=dst_t.tensor,
                                  offset=dst_t.offset + CH * 18,
                                  ap=[list(dst_t.ap[0]), [RUN, 4], [1, RUN]])
                    eng = (nc.sync, nc.scalar, nc.gpsimd)[dx]
                    eng.dma_start(dst, src)

            def conv2_part(chk):
                ic2 = ic2_t[chk % NBUF]
                # ---- conv2: 2 col-packed psum tiles of 8 samples ----
                for tau in range(2):
                    p2 = pp2.tile([128, 1024], F32, tag="c2")
                    for g in range(2):
                        tp = (0, 64 * g) if g else None
                        for dy in range(3):
                            for h in range(2):
                                s0 = 8 * tau + 4 * g + 2 * h
                                mv = bass.AP(
                                    tensor=ic2.tensor,
                                    offset=ic2.offset + s0 * 18
                                    + dy * (CH * 18),
                                    ap=[list(ic2.ap[0]), [18, 2],
                                        [CH * 18, 16], [1, 16]])
                                nc.tensor.matmul(
                                    p2[64 * g:64 * (g + 1),
                                       h * 512:(h + 1) * 512],
                                    w2_sb[:, dy, :], mv,
                                    start=(dy == 0), stop=(dy == 2),
                                    tile_position=tp)
                    # DVE: 2x2 maxpool, one XY-window reduce
                    xm2b = wk.tile([128, 256], F32, tag="xm2b")
                    nc.vector.tensor_reduce(
                        ap_of(xm2b, [[8, 32], [1, 8]]),
                        ap_of(p2, [[32, 32], [2, 8], [16, 2], [1, 2]]),
                        mybir.AxisListType.XY, ALU.max)
                    # ACT sign(bn2) -> +-1 fp8 into s2all
                    sh0 = (2 * chk + tau) * 4
                    nc.scalar.activation(
                        ap_of(s2all, [[1, 4], [1024, 8], [128, 8]], sh0),
                        xm2b[:], AF.Sign, bias=sh2_sb[:], scale=sc2_sb[:])

                # ---- fc weight preloads on Pool during early chunks ----
                if chk == 0:
                    nc.gpsimd.dma_start(fc1w_sb[:], bass.AP(
                        tensor=d_fc1w, offset=0,
                        ap=[[512, 128], [65536, 32], [1, 512]]))
                elif chk == 1:
                    nc.gpsimd.dma_start(fc2w_sb[:], bass.AP(
                        tensor=d_fc2w, offset=0,
                        ap=[[256, 128], [32768, 4], [1, 256]]))
                    nc.gpsimd.dma_start(w3h_sb[:], bass.AP(
                        tensor=d_w3h, offset=0,
                        ap=[[10, 128], [1280, 2], [1, 10]]))
                    nc.gpsimd.dma_start(w3l_sb[:], bass.AP(
                        tensor=d_w3l, offset=0,
                        ap=[[10, 128], [1280, 2], [1, 10]]))
                elif chk == 2:
                    for t, d in [(sc3_sb, d_sc3), (sh3_sb, d_sh3),
                                 (sc4_sb, d_sc4), (sh4_sb, d_sh4),
                                 (b3bc_sb, d_b3bc)]:
                        nc.gpsimd.dma_start(t[:], d.ap())

                # ---- repack waves: one DMA per (g, yxp, wave) ----
                if chk in (NCHUNK // 2 - 1, NCHUNK - 1):
                    w = 0 if chk == NCHUNK // 2 - 1 else 1
                    for g in range(2):
                        for yxp in range(2):
                            src_t = s2all[64 * g:64 * (g + 1)]
                            src = bass.AP(
                                tensor=src_t.tensor,
                                offset=src_t.offset + 128 * yxp + 64 * w,
                                ap=[list(src_t.ap[0]), [256, 32], [1, 64]])
                            dst_t = fc1in[64 * yxp:64 * (yxp + 1)]
                            dst = bass.AP(
                                tensor=dst_t.tensor,
                                offset=dst_t.offset + g * 128 + 64 * w,
                                ap=[list(dst_t.ap[0]), [256, 32], [1, 64]])
                            eng = (nc.sync, nc.scalar, nc.gpsimd)[
                                (2 * g + yxp) % 3]
                            eng.dma_start(dst, src)

            conv1_part(0)
            for chk in range(1, NCHUNK):
                conv1_part(chk)
                conv2_part(chk - 1)
            conv2_part(NCHUNK - 1)

            # ================= fc phase =================
            # fc1: 4 m-tiles, fp8 DoubleRow over kt pairs
            for m in range(4):
                pf = pp1.tile([128, 512], F32, tag="c1")
                for kt in range(16):
                    lw = bass.AP(
                        tensor=fc1w_sb.tensor,
                        offset=fc1w_sb.offset + 2 * kt * 512 + 128 * m,
                        ap=[list(fc1w_sb.ap[0]), [512, 2], [1, 128]])
                    mv = bass.AP(
                        tensor=fc1in.tensor,
                        offset=fc1in.offset + 2 * kt * BC,
                        ap=[list(fc1in.ap[0]), [BC, 2], [1, BC]])
                    nc.tensor.matmul(pf[:, :BC], lw, mv,
                                     start=(kt == 0), stop=(kt == 15),
                                     perf_mode=mybir.MatmulPerfMode.DoubleRow)
                nc.scalar.activation(s3_sb[:, m, :], pf[:, :BC], AF.Sign,
                                     bias=sh3_sb[:, m:m + 1],
                                     scale=sc3_sb[:, m:m + 1])

            # fc2: 2 m-tiles, fp8 DoubleRow over kt pairs
            for m2 in range(2):
                pg = pp1.tile([128, 512], F32, tag="c1")
                for kt in range(2):
                    lw = bass.AP(
                        tensor=fc2w_sb.tensor,
                        offset=fc2w_sb.offset + 2 * kt * 256 + 128 * m2,
                        ap=[list(fc2w_sb.ap[0]), [256, 2], [1, 128]])
                    mv = bass.AP(
                        tensor=s3_sb.tensor,
                        offset=s3_sb.offset + 2 * kt * BC,
                        ap=[list(s3_sb.ap[0]), [BC, 2], [1, BC]])
                    nc.tensor.matmul(pg[:, :BC], lw, mv,
                                     start=(kt == 0), stop=(kt == 1),
                                     perf_mode=mybir.MatmulPerfMode.DoubleRow)
                nc.scalar.activation(u4_sb[:, m2, :], pg[:, :BC], AF.Identity,
                                     bias=sh4_sb[:, m2:m2 + 1],
                                     scale=sc4_sb[:, m2:m2 + 1])

            # clip to [-1,1] in one dual-op DVE instruction
            nc.vector.tensor_scalar(u4_sb[:], u4_sb[:], 1.0, -1.0,
                                    ALU.min, ALU.max)
            # hi/lo split of s4 (DVE casts; keeps ACT free for Exp/Ln)
            nc.vector.tensor_copy(s4h_sb[:], u4_sb[:])
            nc.vector.tensor_sub(s4r_sb[:], u4_sb[:], s4h_sb[:])
            nc.vector.tensor_copy(s4l_sb[:], s4r_sb[:])

            # fc3 + log_softmax; batch tile bt == g block of fc1 columns.
            h3s, mxs, negs, ses, lss = [], [], [], [], []
            for bt in range(2):
                ph = pp2.tile([128, 1024], F32, tag="c2")
                mms = []
                for kt in range(2):
                    lh = s4h_sb[:, kt, 128 * bt:128 * (bt + 1)]
                    ll = s4l_sb[:, kt, 128 * bt:128 * (bt + 1)]
                    mms += [(lh, w3h_sb[:, kt, :]), (ll, w3h_sb[:, kt, :]),
                            (lh, w3l_sb[:, kt, :])]
                for i, (lhs, rhs) in enumerate(mms):
                    nc.tensor.matmul(ph[:, :10], lhs, rhs,
                                     start=(i == 0), stop=(i == len(mms) - 1))
                h3 = wk.tile([128, 10], F32, tag="h3", name=f"h3_{bt}")
                nc.vector.tensor_add(h3[:], ph[:, :10], b3bc_sb[:])
                mx = wk.tile([128, 1], F32, tag="mx", name=f"mx_{bt}")
                nc.vector.tensor_reduce(mx[:], h3[:], mybir.AxisListType.X,
                                        ALU.max)
                negmx = wk.tile([128, 1], F32, tag="negmx", name=f"negmx_{bt}")
                nc.vector.tensor_scalar_mul(negmx[:], mx[:], -1.0)
                h3s.append(h3)
                mxs.append(mx)
                negs.append(negmx)
            for bt in range(2):     # both Exps together (one act table set)
                et = wk.tile([128, 10], F32, tag="et", name=f"et_{bt}")
                se = wk.tile([128, 1], F32, tag="se", name=f"se_{bt}")
                nc.scalar.activation(et[:], h3s[bt][:], AF.Exp,
                                     bias=negs[bt][:], scale=1.0,
                                     accum_out=se[:])
                ses.append(se)
            for bt in range(2):     # then both Lns (single table reload)
                ls = wk.tile([128, 1], F32, tag="ls", name=f"ls_{bt}")
                nc.scalar.activation(ls[:], ses[bt][:], AF.Ln)
                lss.append(ls)
            for bt in range(2):
                tt = wk.tile([128, 1], F32, tag="tt", name=f"tt_{bt}")
                nc.vector.tensor_add(tt[:], mxs[bt][:], lss[bt][:])
                o = wk.tile([128, 10], F32, tag="o", name=f"o_{bt}")
                nc.vector.tensor_scalar_sub(o[:], h3s[bt][:], tt[:])
                # un-permute: psum partition p -> sample row
                # s = 8*(p//4) + 4*bt + p%4
                dst = bass.AP(tensor=d_out, offset=bt * 40,
                              ap=[[80, 32], [10, 4], [1, 10]])
                nc.sync.dma_start(dst, o[:])

    nc.compile()
    return nc


def _get_module():
    if "nc" not in _nc_cache:
        _nc_cache["nc"] = _build_module()
    return _nc_cache["nc"]


def kernel(**inputs):
    from concourse.bass_utils import run_bass_kernel_spmd

    in_maps = _host_prep(inputs)
    nc = _get_module()
    res = run_bass_kernel_spmd(nc, in_maps, core_ids=list(range(NCORES)))
    out = np.concatenate([r["out"] for r in res.results], axis=0)
    return out.astype(np.float32)
